# revision 33
# baseline (speedup 1.0000x reference)
"""Trainium2 Bass kernel for the 2-layer LSTM encoder/decoder problem.

Strategy (8 NeuronCores), V3:
  - Tensor-parallel shard of the 4L=8192 gate rows: core k owns rows
    [256k:256k+256) of each gate (i,f,g,o) -> 1024 gate rows / core.
  - Activations live transposed [feature, batch] on device; batch = 32
    (the two independent scan chains of the reference are batched).
  - Non-autoregressive phases are wavefronted layer-by-layer; the Wih
    contribution for all 4 timesteps is one M=128 bulk matmul.
  - All four decoder matrices SBUF-resident in bf16 (no AR streaming).
  - Trace-driven scheduling (V2+V3):
      * weight loads and AllGather stage/unstage DMAs split across the
        three DMA-capable queues (sync/gpsimd/scalar).
      * two warmup AllGathers at t=0 absorb the ~40us CC init.
      * bias / bulk-U addends folded into the matmul accumulation group
        via an identity-matrix matmul (keeps them off the DVE chain).
      * decoder bulk-U and weight loads overlap the encoder phase; the
        whh-pool rotation is ordered so no load waits on a late-freed
        buffer.
      * matmul groups emit [indep stream][addend][filler][dep stream]
        so independent work (incl. the final conv, spread over 4 AR
        steps) fills the AllGather windows.
"""

import tempfile

import numpy as np
import ml_dtypes

import concourse.bass as bass
import concourse.bacc as bacc
import concourse.mybir as mybir
import concourse.tile as tile
from concourse import bass_utils

# Problem constants (hardcoded per contract)
C, H, W = 512, 4, 4
SPLIT, PRED = 4, 4
L = 2048           # lstm feature size
B = 16             # reference batch
NB = 32            # device batch (two chains)
NCORES = 8
SL = L // NCORES   # 256: hidden slice per core
GL = 4 * SL        # 1024: gate rows per core
NT = L // 128      # 16 k-tiles
NAR = PRED + SPLIT - 1  # 7 autoregressive steps

F32 = mybir.dt.float32
BF16 = mybir.dt.bfloat16
NPBF = ml_dtypes.bfloat16

# Permutation: device feature f' = 512*h + c  <->  natural f = 4*c + h
PERM = np.array([4 * (f % C) + f // C for f in range(L)], dtype=np.int64)
IPERM = np.argsort(PERM)

_CACHE = {}


def _build_nc():
    nc = bacc.Bacc("TRN2", target_bir_lowering=False, debug=False,
                   num_devices=NCORES)

    def din(name, shape, dt=F32):
        return nc.dram_tensor(name, shape, dt, kind="ExternalInput").ap()

    def dout(name, shape, dt=F32):
        return nc.dram_tensor(name, shape, dt, kind="ExternalOutput").ap()

    xET = din("xET", [128, NT * 4 * NB], BF16)
    xDT = din("xDT", [128, NT * 4 * NB], BF16)
    eWih = din("eWih", [2, L, GL], BF16)
    eWhh = din("eWhh", [2, L, GL], BF16)
    dWih = din("dWih", [2, L, GL], BF16)
    dWhh = din("dWhh", [2, L, GL], BF16)
    eB = din("eB", [2, 32, GL], BF16)   # bias replicated over 32 partitions
    dB = din("dB", [2, 32, GL], BF16)
    eye = din("eye", [128, 128], BF16)  # identity (addend matmul trick)
    cWT = din("cWT", [2 * C, C], BF16)  # conv_W.T
    cB = din("cB", [64, C])             # conv bias replicated over 64 rows

    # per-core own h2 slice [32 batch, 256 feat] per chunk; host gathers
    chunks_out = dout("chunks_out", [8, NB, SL], BF16)
    convout = dout("convout", [4, 64, C])   # [w, (h,b), out_ch]

    with tile.TileContext(nc) as tc:
        with (
            tc.tile_pool(name="bias", bufs=2) as biasp,
            tc.tile_pool(name="whh", bufs=4) as whhp,
            tc.tile_pool(name="cwt", bufs=1) as cwtp,
            tc.tile_pool(name="wstr", bufs=3) as wstrp,
            tc.tile_pool(name="x2t", bufs=1) as x2tp,
            tc.tile_pool(name="xin", bufs=2) as xinp,
            tc.tile_pool(name="usb", bufs=2) as usbp,
            tc.tile_pool(name="ut", bufs=6) as utp,
            tc.tile_pool(name="eye", bufs=1) as eyep,
            tc.tile_pool(name="h2big", bufs=7) as h2bigp,
            tc.tile_pool(name="h1big", bufs=2) as h1bigp,
            tc.tile_pool(name="gw", bufs=1) as gwp,
            tc.tile_pool(name="cst", bufs=2) as cstp,
            tc.tile_pool(name="hsl", bufs=1) as hslp,
            tc.tile_pool(name="psu", bufs=2, space="PSUM") as psup,
            tc.tile_pool(name="psg", bufs=4, space="PSUM") as psgp,
            tc.tile_pool(name="psd", bufs=2, space="PSUM") as psdp,
            tc.tile_pool(name="dram", bufs=3, space="DRAM") as dramp,
        ):
            QUEUES = [nc.sync, nc.gpsimd, nc.scalar]

            # ---- warmup collective: kicks off the ~45us CC init at t=0
            # (without an early collective enqueued, CC init lazily
            # completes only at ~160us and the first real AllGather
            # stalls the whole pipeline behind it)
            with tc.high_priority():
                warm = gwp.tile([32, 32], BF16, tag="warm")
                nc.gpsimd.memset(warm[:], 0)
                cin_w = dramp.tile([32, 32], BF16, tag="warmin")
                nc.gpsimd.dma_start(cin_w, warm[:])
                cout_w = dramp.tile([256, 32], BF16, tag="warmout",
                                    addr_space="Shared")
                nc.gpsimd.collective_compute(
                    "AllGather", mybir.AluOpType.bypass,
                    replica_groups=[list(range(NCORES))],
                    ins=[cin_w[:]], outs=[cout_w[:]],
                )

            def load_bias(src, l, name):
                t_ = biasp.tile([32, GL], BF16, tag="bias", name=name)
                nc.gpsimd.dma_start(t_[:], src[l])
                return t_

            def alloc_w(name):
                # two half-tiles (kt 0-7 / 8-15): WAR release and load
                # consumption at half granularity, so a decoder matrix
                # can be consumed while its second half still loads
                return tuple(
                    whhp.tile([128, (NT // 2) * GL], BF16, tag=f"whh{h}",
                              name=f"{name}_{h}") for h in range(2))

            def load_w(w_dram, l, name, kts=None, wt=None):
                # fine-grained per-kt DMAs (256KB each) alternating
                # queues so chain DMAs never sit behind a big transfer
                if wt is None:
                    wt = alloc_w(name)
                sr = w_dram[l].rearrange("(kt p) n -> p kt n", p=128)
                for kt in (range(NT) if kts is None else kts):
                    h, ko = kt // 8, kt % 8
                    wr = wt[h][:].rearrange("p (kt n) -> p kt n", kt=8)
                    eng = nc.scalar if kt % 2 == 0 else nc.gpsimd
                    eng.dma_start(wr[:, ko:ko + 1], sr[:, kt:kt + 1])
                return wt

            def ag(ht):
                """AllGather this core's [256, 32] h slice -> [2048, 32]."""
                cin = dramp.tile([2 * 128, NB], BF16, tag="agin")
                cr = cin.rearrange("(q j) b -> j q b", j=32)
                hr = ht[:].rearrange("j (q b) -> j q b", b=NB)
                nc.sync.dma_start(cr[:, 0:4], hr[:, 0:4])
                nc.gpsimd.dma_start(cr[:, 4:8], hr[:, 4:8])
                cout = dramp.tile([L, NB], BF16, tag="agout",
                                  addr_space="Shared")
                nc.gpsimd.collective_compute(
                    "AllGather", mybir.AluOpType.bypass,
                    replica_groups=[list(range(NCORES))],
                    ins=[cin[:]], outs=[cout[:]],
                )
                return cout

            def big_from_ag(cout, pool, tag):
                # 4 interleaved sub-tiles: sub q holds kts {q, q+4, q+8,
                # q+12} as [p, ktg*32+b]. Per-tile deps let the gated
                # matmul stream start when its first sub-tile lands
                # instead of waiting for the whole unstage.
                sr4 = cout.rearrange("(ktg q p) b -> q p ktg b",
                                     p=128, q=4)
                subs = []
                for q in range(4):
                    st_ = pool.tile([128, (NT // 4) * NB], BF16,
                                    tag=f"{tag}{q}", name=f"{tag}{q}")
                    eng = nc.sync if q % 2 == 0 else nc.gpsimd
                    eng.dma_start(
                        st_[:].rearrange("p (ktg b) -> p ktg b", ktg=4),
                        sr4[q])
                    subs.append(st_)
                return subs

            def x2t_from_ag(cout, x2t, t):
                # write h1T of step t into x2t sub-tile q at [p, ktg,t,b]
                sr4 = cout.rearrange("(ktg q p) b -> q p ktg b",
                                     p=128, q=4)
                for q in range(4):
                    br = x2t[q][:].rearrange(
                        "p (ktg t b) -> p ktg t b", ktg=4, t=4)[:, :, t, :]
                    eng = nc.sync if q % 2 == 0 else nc.gpsimd
                    eng.dma_start(br, sr4[q])

            SIG = mybir.ActivationFunctionType.Sigmoid
            TANH = mybir.ActivationFunctionType.Tanh

            def warm_fill(n):
                # dummy matmuls that keep the PE clock at full p-state
                # through an AllGather window (the post-gap matmuls
                # otherwise run ~2x slow for ~3us while ramping)
                for _ in range(n):
                    dt_ = psdp.tile([32, 512], F32, tag="psd")
                    nc.tensor.matmul(dt_[:], eye_sb[0:32, 0:32],
                                     eb0[:, 0:512], start=True, stop=True,
                                     skip_group_check=True)

            def matgroup(streams, addend=None, name="", filler=None,
                         dummy_n=0):
                """One accumulation group -> psum pair [32,512]x2.
                Emission order: streams[0], addend, filler, streams[1:]
                (addend first if <=1 stream) so independent work fills
                the AllGather window before the gated stream."""
                psums = [psgp.tile([32, 512], F32, tag="psg",
                                   name=f"psg{name}{n_}") for n_ in range(2)]
                started = [False, False]

                def mm(n, lhsT, rhs, stop):
                    nc.tensor.matmul(psums[n][:], lhsT, rhs,
                                     start=not started[n], stop=stop,
                                     skip_group_check=True)
                    started[n] = True

                def emit_stream(lhs_fn, w_sb, last):
                    for kt in range(NT):
                        lhs = lhs_fn(kt)
                        wh = w_sb[kt // 8]
                        base = (kt % 8) * GL
                        for n in range(2):
                            mm(n, lhs,
                               wh[:, base + n * 512:base + n * 512 + 512],
                               last and kt == NT - 1)

                def emit_addend(last):
                    for n in range(2):
                        mm(n, eye_sb[0:32, 0:32],
                           addend[:, n * 512:n * 512 + 512], last)

                n_items = len(streams) + (addend is not None)
                idx = 0
                if len(streams) <= 1:
                    if addend is not None:
                        idx += 1
                        emit_addend(idx == n_items)
                    if filler is not None:
                        filler()
                    if dummy_n:
                        warm_fill(dummy_n)
                    if streams:
                        idx += 1
                        emit_stream(*streams[0], idx == n_items)
                else:
                    idx += 1
                    emit_stream(*streams[0], False)
                    if addend is not None:
                        idx += 1
                        emit_addend(idx == n_items)
                    if filler is not None:
                        filler()
                    if dummy_n:
                        warm_fill(dummy_n)
                    for s in streams[1:]:
                        idx += 1
                        emit_stream(*s, idx == n_items)
                return psums

            def cell(psums, c_old, ltag, out_idx=None):
                """LSTM cell elementwise from psum pair (i,f | g,o).
                Returns (c_new, ht), ht = bf16 block-transposed [32, SL]."""
                act = nc.scalar.activation
                if_t = gwp.tile([32, 2 * SL], F32, tag="ift")
                gt_t = gwp.tile([32, SL], F32, tag="gtt")
                o_t = gwp.tile([32, SL], F32, tag="ot")
                act(if_t[:], psums[0][:], SIG)            # i, f fused
                act(gt_t[:], psums[1][:, 0:SL], TANH)
                act(o_t[:], psums[1][:, SL:2 * SL], SIG)
                c_new = cstp.tile([32, SL], F32, tag="c" + ltag)
                tmp = gwp.tile([32, SL], F32, tag="tmp")
                if c_old is not None:
                    cmul = gwp.tile([32, SL], F32, tag="cmul")
                    nc.vector.tensor_mul(cmul[:], if_t[:, SL:2 * SL],
                                         c_old[:])
                    nc.vector.tensor_mul(tmp[:], if_t[:, 0:SL], gt_t[:])
                    nc.vector.tensor_add(c_new[:], cmul[:], tmp[:])
                else:
                    nc.vector.tensor_mul(tmp[:], if_t[:, 0:SL], gt_t[:])
                    nc.vector.tensor_copy(c_new[:], tmp[:])
                tanh_c = gwp.tile([32, SL], F32, tag="tanhc")
                act(tanh_c[:], c_new[:], TANH)
                hb = gwp.tile([32, SL], BF16, tag="hb")
                nc.vector.tensor_mul(hb[:], o_t[:], tanh_c[:])
                if out_idx is not None:
                    nc.gpsimd.dma_start(chunks_out[out_idx], hb[:])
                ht = hslp.tile([32, SL], BF16, tag="hsl")
                nc.vector.transpose(ht[:], hb[:])
                return c_new, ht

            def bulk_start(name, b128):
                psums = [psup.tile([128, 512], F32, tag="psu",
                                   name=f"psu{name}{n_}") for n_ in range(2)]
                for n in range(2):
                    nc.tensor.matmul(
                        psums[n][:], eye_sb[:, :],
                        b128[:, n * 512:n * 512 + 512],
                        start=True, stop=False, skip_group_check=True)
                return psums

            def bulk_chunk(psums, lhs_fn, w_dram, l, kts):
                for kt in kts:
                    wt = wstrp.tile([128, GL], BF16, tag="wstr")
                    eng = nc.scalar if kt % 2 == 0 else nc.gpsimd
                    eng.dma_start(
                        wt[:], w_dram[l, kt * 128:(kt + 1) * 128, :])
                    lhs = lhs_fn(kt)
                    for n in range(2):
                        nc.tensor.matmul(
                            psums[n][:], lhs, wt[:, n * 512:(n + 1) * 512],
                            start=False, stop=(kt == NT - 1),
                            skip_group_check=True)

            def bulk_finish(psums, name):
                u_sb = usbp.tile([128, GL], BF16, tag="usb", name=name)
                for n in range(2):
                    nc.vector.tensor_copy(u_sb[:, n * 512:(n + 1) * 512],
                                          psums[n][:])
                uts = [u_sb[0:32, :]]
                for t in range(1, 4):
                    ut = utp.tile([32, GL], BF16, tag="ut")
                    nc.sync.dma_start(ut[:], u_sb[32 * t:32 * t + 32, :])
                    uts.append(ut[:])
                return uts

            def bulk_u(lhs_fn, w_dram, l, b128, name):
                psums = bulk_start(name, b128)
                bulk_chunk(psums, lhs_fn, w_dram, l, range(NT))
                return bulk_finish(psums, name)

            def rep_bias(b32, name):
                # replicate [32, GL] bias to [128, GL] (4 t-blocks)
                b128 = biasp.tile([128, GL], BF16, tag="b128", name=name)
                for t in range(4):
                    nc.gpsimd.dma_start(b128[32 * t:32 * t + 32, :], b32[:])
                return b128

            def load_xin(x_dram, name):
                # host pre-laid-out [p, kt*128 + t*32 + b]; 2-queue split
                t_ = xinp.tile([128, NT * 128], BF16, tag="xin", name=name)
                half = NT * 64
                nc.scalar.dma_start(t_[:, 0:half], x_dram[:, 0:half])
                nc.gpsimd.dma_start(t_[:, half:2 * half],
                                    x_dram[:, half:2 * half])
                return t_

            def hblk(subs, kt):
                q, ktg = kt % 4, kt // 4
                return subs[q][:, ktg * NB:ktg * NB + NB]

            def x2t_block(x2t, kt, t):
                q, ktg = kt % 4, kt // 4
                base = ktg * 128 + 32 * t
                return x2t[q][:, base:base + 32]

            def dual_scan(l0_init_lhs, uts, whh0_fn, whh1_fn, wih1_fn, b1,
                          c1_init, c2_init, x2t_out, h2_init_big,
                          zero_init, store_de, post_step=None,
                          pre_l1=None):
                """Wavefront over both layers: layer-0 (bulk-U + Whh0) and
                layer-1 (step-wise: Wih1 @ h1_t + Whh1 @ h2_{t-1}).
                whh0_fn/whh1_fn/wih1_fn are thunks returning the resident
                tiles (so loads can be emitted lazily via hooks)."""
                c1p, c2p = c1_init, c2_init
                h2_prev = h2_init_big
                for t in range(4):
                    # ---- layer 0 step t ----
                    if zero_init and t == 0:
                        psums = matgroup([], addend=uts[0], name="e0")
                        c1p, ht = cell(psums, None, "1")
                    else:
                        if t == 0:
                            lhs = l0_init_lhs
                        else:
                            lhs = lambda kt, tt=t: x2t_block(
                                x2t_out, kt, tt - 1)
                        psums = matgroup([(lhs, whh0_fn())], addend=uts[t],
                                         name="l0")
                        c1p, ht = cell(psums, c1p, "1")
                    cout = ag(ht)
                    x2t_from_ag(cout, x2t_out, t)
                    if pre_l1 is not None:
                        pre_l1(t)
                    # ---- layer 1 step t ----
                    wih_lhs = lambda kt, tt=t: x2t_block(x2t_out, kt, tt)
                    if zero_init and t == 0:
                        psums = matgroup([(wih_lhs, wih1_fn())], addend=b1,
                                         name="e1")
                        c2p, ht = cell(psums, None, "2")
                    else:
                        h2b = h2_prev
                        psums = matgroup(
                            [(lambda kt, h=h2b: hblk(h, kt), whh1_fn()),
                             (wih_lhs, wih1_fn())],
                            addend=b1, name="l1")
                        c2p, ht = cell(
                            psums, c2p, "2",
                            out_idx=(0 if store_de and t == 3 else None))
                    cout = ag(ht)
                    h2_prev = big_from_ag(cout, h2bigp, "h2big")
                    if post_step is not None:
                        post_step(t)
                return c1p, c2p, h2_prev

            # =========================================================
            # Preamble: encoder bulk only (CC init runs ~21-70us anyway);
            # all other loads are emitted at their need-time so the
            # ~210GB/s aggregate DMA never blocks an earlier consumer.
            # =========================================================
            eye_sb = eyep.tile([128, 128], BF16, tag="eye")
            nc.sync.dma_start(eye_sb[:], eye[:, :])
            eb0 = load_bias(eB, 0, "eb0")
            eb1 = load_bias(eB, 1, "eb1")
            xe_sb = load_xin(xET, "xe_sb")
            # allocate whh_e0's pool buffers FIRST (rotation: its
            # buffers, freed at enc L0 t3, recycle for wih_d1)
            whh_e0 = alloc_w("whh_e0")
            eb0_128 = rep_bias(eb0[:], "eb0_128")
            uts_e = bulk_u(
                lambda kt: xe_sb[:, kt * 128:(kt + 1) * 128], eWih, 0,
                eb0_128[:], "ue")

            # =========================================================
            # Phase E: encoder (batch 32 = [x2 fwd chain, x1-rev chain])
            # =========================================================
            x2t_e = [x2tp.tile([128, (NT // 4) * 128], BF16,
                                     tag=f"x2t{q}", name=f"x2te{q}")
                     for q in range(4)]

            dec_w = {}
            uts_d = []

            def enc_pre_l1(t):
                # loads ordered by first use: wih_e1 (enc L1 t0) before
                # whh_e0 (enc L0 t1) before whh_e1 (enc L1 t1); the dec
                # bulk-U is emitted in 4kt chunks so its matmuls fill
                # four separate enc AllGather windows
                if t == 0:
                    dec_w["wih_e1"] = load_w(eWih, 1, "wih_e1")
                elif t == 1:
                    dec_w["whh_e1"] = load_w(eWhh, 1, "whh_e1")
                elif t == 2:
                    bulk_chunk(dec_w["psu_d"], dec_w["xd_lhs"], dWih, 0,
                               range(4, 8))
                elif t == 3:
                    bulk_chunk(dec_w["psu_d"], dec_w["xd_lhs"], dWih, 0,
                               range(12, 16))
                    uts_d.extend(bulk_finish(dec_w["psu_d"], "ud"))

            def enc_post(t):
                if t == 0:
                    load_w(eWhh, 0, "whh_e0", wt=whh_e0)
                elif t == 1:
                    # decoder prep; bulk-U start + first chunk
                    db0 = load_bias(dB, 0, "db0")
                    db1 = load_bias(dB, 1, "db1")
                    dec_w["db0"], dec_w["db1"] = db0, db1
                    xd_sb = load_xin(xDT, "xd_sb")
                    db0_128 = rep_bias(db0[:], "db0_128")
                    dec_w["xd_lhs"] = (
                        lambda kt: xd_sb[:, kt * 128:(kt + 1) * 128])
                    dec_w["psu_d"] = bulk_start("ud", db0_128[:])
                    bulk_chunk(dec_w["psu_d"], dec_w["xd_lhs"], dWih, 0,
                               range(0, 4))
                    dec_w["whh_d0"] = load_w(dWhh, 0, "whh_d0")
                elif t == 2:
                    bulk_chunk(dec_w["psu_d"], dec_w["xd_lhs"], dWih, 0,
                               range(8, 12))
                    # reuses whh_e0's buffer (freed at enc L0 t3)
                    dec_w["wih_d1"] = load_w(dWih, 1, "wih_d1")
                elif t == 3:
                    # reuses wih_e1's buffer (freed at enc L1 t3)
                    dec_w["whh_d1"] = load_w(dWhh, 1, "whh_d1")

            c_e1, c_e2, h2_big = dual_scan(
                None, uts_e, lambda: whh_e0,
                lambda: dec_w["whh_e1"], lambda: dec_w["wih_e1"], eb1[:],
                None, None, x2t_e, None, True, False,
                post_step=enc_post, pre_l1=enc_pre_l1)

            # =========================================================
            # Phase D1: decoder consume (batch = [x1 fwd, x2-rev])
            # =========================================================
            def dec_post(t):
                # wih_d0 (needed only at AR t0) trickles in across the
                # dec scan, reusing wih_e1's buffer (freed enc L1 t3)
                if t == 0:
                    dec_w["wih_d0"] = load_w(dWih, 0, "wih_d0",
                                             kts=range(0, 4))
                else:
                    load_w(dWih, 0, "wih_d0", kts=range(4 * t, 4 * t + 4),
                           wt=dec_w["wih_d0"])

            x2t_d = [x2tp.tile([128, (NT // 4) * 128], BF16,
                                     tag=f"x2t{q}", name=f"x2td{q}")
                     for q in range(4)]
            db0, db1 = dec_w["db0"], dec_w["db1"]
            c1, c2, h2_big = dual_scan(
                lambda kt: x2t_block(x2t_e, kt, 3), uts_d,
                lambda: dec_w["whh_d0"], lambda: dec_w["whh_d1"],
                lambda: dec_w["wih_d1"], db1[:],
                c_e1, c_e2, x2t_d, h2_big, False, True, post_step=dec_post)
            wih_d0 = dec_w["wih_d0"]
            whh_d0 = dec_w["whh_d0"]
            whh_d1 = dec_w["whh_d1"]
            wih_d1 = dec_w["wih_d1"]

            # =========================================================
            # Phase D2: autoregressive decoder (7 steps, zero streaming)
            # =========================================================
            conv_tiles = {"de": h2_big}
            h1_big = None
            cb_sb = biasp.tile([64, C], F32, tag="cbias", name="cb_sb",
                               bufs=1)
            nc.gpsimd.dma_start(cb_sb[:], cB[:])
            cwt_sb = cwtp.tile([128, 8 * C], BF16, tag="cwt")
            nc.scalar.dma_start(
                cwt_sb[:].rearrange("p (j o) -> p j o", j=8),
                cWT.rearrange("(j p) o -> p j o", p=128))

            conv_sts = {}
            conv_pcv = {}

            def emit_conv_copies(w):
                # pre-stage conv lhs tiles on the DVE during the matmul
                # window before the filler needs them (cvl bufs=8)
                b1s = [conv_tiles["de"], conv_tiles["ar0"],
                       conv_tiles["ar1"], conv_tiles["ar2"]]
                b2s = [conv_tiles["ar2"], conv_tiles["ar1"],
                       conv_tiles["ar0"], conv_tiles["de"]]
                sts = []
                for br, src in ((0, b1s[w]), (1, b2s[w])):
                    for j in range(4):
                        lhs = src[j][:].rearrange(
                            "p (ktg b) -> p ktg b", ktg=4)
                        st = gwp.tile([128, 64], BF16, tag="cvl", bufs=8,
                                      name=f"cvl{w}_{br}_{j}")
                        nc.vector.tensor_copy(
                            st[:].rearrange("p (h b) -> p h b", h=4),
                            lhs[:, :, 16 * br:16 * br + 16])
                        sts.append(st)
                conv_sts[w] = sts

            def emit_conv_mms(w, half):
                # AllGather-window filler: 4 independent matmuls per half
                if half == 0:
                    conv_pcv[w] = psup.tile([128, 512], F32, tag="psu",
                                            name=f"pcv{w}")
                pcv = conv_pcv[w]
                for i in range(4 * half, 4 * half + 4):
                    nc.tensor.matmul(
                        pcv[0:64, :], conv_sts[w][i][:],
                        cwt_sb[:, i * C:(i + 1) * C],
                        start=(i == 0), stop=(i == 7),
                        skip_group_check=True)

            def emit_conv_tail(w):
                pcv = conv_pcv[w]
                cvs = gwp.tile([64, C], F32, tag="g", name=f"cvs{w}")
                nc.vector.tensor_add(cvs[:], pcv[0:64, :], cb_sb[:])
                cvo = gwp.tile([64, C], F32, tag="g2", name=f"cvo{w}")
                nc.vector.tensor_scalar_mul(cvo[:], cvs[:], 0.2)
                nc.vector.tensor_max(cvo[:], cvo[:], cvs[:])
                nc.gpsimd.dma_start(convout[w], cvo[:])

            for t in range(NAR):
                if 3 <= t <= 6:
                    emit_conv_copies(t - 3)
                h2b, h1b = h2_big, h1_big
                if t == 0:
                    l0_lhs = lambda kt: x2t_block(x2t_d, kt, 3)
                else:
                    l0_lhs = lambda kt, h=h1b: hblk(h, kt)
                filler = None
                if 3 <= t <= 6:
                    filler = lambda w=t - 3: emit_conv_mms(w, 0)
                psums = matgroup(
                    [(l0_lhs, whh_d0),
                     (lambda kt, h=h2b: hblk(h, kt), wih_d0)],
                    addend=db0[:], name="a0", filler=filler)
                c1, ht = cell(psums, c1, "1")
                cout = ag(ht)
                h1_big = big_from_ag(cout, h1bigp, "h1big")

                h1b2 = h1_big
                filler1 = None
                if 3 <= t <= 6:
                    filler1 = lambda w=t - 3: emit_conv_mms(w, 1)
                psums = matgroup(
                    [(lambda kt, h=h2b: hblk(h, kt), whh_d1),
                     (lambda kt, h=h1b2: hblk(h, kt), wih_d1)],
                    addend=db1[:], name="a1", filler=filler1)
                c2, ht = cell(psums, c2, "2", out_idx=t + 1)
                if 3 <= t <= 6:
                    emit_conv_tail(t - 3)
                if t < NAR - 1:
                    cout = ag(ht)
                    h2_big = big_from_ag(cout, h2bigp, "h2big")
                    if t < 3:
                        conv_tiles[f"ar{t}"] = h2_big

    nc.compile()
    return nc


def _prep_inputs(x1, x2, enc_Wih, enc_Whh, enc_bih, enc_bhh,
                 dec_Wih, dec_Whh, dec_bih, dec_bhh, conv_W, conv_b):
    def colvecs(x):
        return [np.ascontiguousarray(x[:, :, :, t].reshape(B, L))
                for t in range(4)]

    x1c, x2c = colvecs(x1), colvecs(x2)

    def ximg(xa):
        # [4, L, NB] -> SBUF image [128, kt*128 + t*32 + b]
        return np.ascontiguousarray(
            xa.reshape(4, NT, 128, NB).transpose(2, 1, 0, 3)
            .reshape(128, NT * 4 * NB)).astype(NPBF)

    xET = ximg(np.stack([
        np.concatenate([x2c[t], x1c[3 - t]], axis=0)[:, PERM].T
        for t in range(4)]))
    xDT = ximg(np.stack([
        np.concatenate([x1c[t], x2c[3 - t]], axis=0)[:, PERM].T
        for t in range(4)]))

    def prep_core(k, Wih, Whh, bih, bhh):
        rows = np.concatenate([g * L + PERM[k * SL:(k + 1) * SL]
                               for g in range(4)])
        wihT = np.stack([np.ascontiguousarray(Wih[l][rows][:, PERM].T)
                         for l in range(2)])
        whhT = np.stack([np.ascontiguousarray(Whh[l][rows][:, PERM].T)
                         for l in range(2)])
        bb = np.stack([(bih[l] + bhh[l])[rows] for l in range(2)])
        brep = np.broadcast_to(bb[:, None, :], (2, 32, GL)).copy()
        return wihT.astype(NPBF), whhT.astype(NPBF), brep.astype(NPBF)

    cWT = np.ascontiguousarray(conv_W.T).astype(NPBF)
    cBr = np.broadcast_to(conv_b[None, :], (64, C)).copy().astype(np.float32)
    eye = np.eye(128, dtype=NPBF)

    in_maps = []
    for k in range(NCORES):
        eWihT, eWhhT, eBr = prep_core(k, enc_Wih, enc_Whh, enc_bih, enc_bhh)
        dWihT, dWhhT, dBr = prep_core(k, dec_Wih, dec_Whh, dec_bih, dec_bhh)
        in_maps.append({
            "xET": xET, "xDT": xDT,
            "eWih": eWihT, "eWhh": eWhhT, "eB": eBr,
            "dWih": dWihT, "dWhh": dWhhT, "dB": dBr,
            "eye": eye, "cWT": cWT, "cB": cBr,
        })
    return in_maps


def _postprocess(results, x1, x2):
    # gather chunk slices across cores: core k owns features [256k:256k+256)
    chunks = np.zeros((8, B * 2, L), np.float32)
    for k in range(NCORES):
        chunks[:, :, k * SL:(k + 1) * SL] = np.asarray(
            results[k]["chunks_out"], dtype=np.float32)
    convout = results[0]["convout"]

    def tochunk(t, half):
        v = chunks[t, half * B:(half + 1) * B, :]   # [16, L] dev order
        return v[:, IPERM].reshape(B, C, H)

    de1 = tochunk(0, 0)
    p1 = [tochunk(1 + j, 0) for j in range(NAR)]
    de2 = tochunk(0, 1)
    p2 = [tochunk(1 + j, 1) for j in range(NAR)]

    mid1 = np.stack([de1, p1[0], p1[1], p1[2]], axis=-1)
    tail1 = np.stack([p1[3], p1[4], p1[5], p1[6]], axis=-1)
    head2 = np.stack([p2[6], p2[5], p2[4], p2[3]], axis=-1)
    mid2 = np.stack([p2[2], p2[1], p2[0], de2], axis=-1)

    out = convout.reshape(4, 4, B, C).transpose(2, 3, 1, 0)
    out = np.ascontiguousarray(out, dtype=np.float32)
    return (out, np.asarray(x1), mid1, tail1, head2, mid2, np.asarray(x2))


def _run(in_maps, trace=False):
    if "nc" not in _CACHE:
        _CACHE["nc"] = _build_nc()
        _CACHE["tmpdir"] = tempfile.mkdtemp(prefix="lstmk_")
    nc = _CACHE["nc"]
    res = bass_utils.run_bass_kernel_spmd(
        nc, in_maps, core_ids=list(range(NCORES)), trace=trace,
        tmpdir=_CACHE["tmpdir"] if trace else None)
    return res


def kernel(**inputs):
    inputs = {k: np.asarray(v, dtype=np.float32) for k, v in inputs.items()}
    in_maps = _prep_inputs(**inputs)
    if "warm" not in _CACHE:
        # discard the first execution after compile: shields against a
        # cold-start artifact observed on the very first device run
        _run(in_maps, trace=False)
        _CACHE["warm"] = True
    res = _run(in_maps, trace=False)
    return _postprocess(res.results, inputs["x1"], inputs["x2"])


def kernel_traced(**inputs):
    inputs = {k: np.asarray(v, dtype=np.float32) for k, v in inputs.items()}
    in_maps = _prep_inputs(**inputs)
    res = _run(in_maps, trace=True)
    return _postprocess(res.results, inputs["x1"], inputs["x2"]), res
